# revision 26
# baseline (speedup 1.0000x reference)
"""Trainium2 Bass kernel for nn_Net_60413009985719.

Reference semantics: x[L] -> 5 stacked single-step LSTM cells (seq_len=1,
zero initial (h, c)) applied independently to every "batch" row, then the
head reads ONLY h[-1:].  Because h_prev = c_prev = 0, rows never interact:
the output depends solely on the scalar x[L-1].

Sharding: data-parallel over L across the 8 cores, as per the spec hint --
"only the shard owning the last row" has live work.  Core 7 owns
x[437500:500000] under the natural row split, so core 7 runs the whole
5-cell + MLP-head chain; cores 0..6 hold dead rows (their h values are
never read by the head) and branch straight to the program end.  Each
core receives a per-core `flag` input (1 only on core 7) and the whole
compute body sits inside an `If(flag == 1)` branch.

Live-core implementation (unchanged from the tuned single-core version):
- Every matvec is a K=65 PE matmul with the bias folded in as an extra
  contraction row against a constant 1.0 in the rhs vector.  The f-gate is
  dead (f * c_prev == 0) and is never computed.
- The whole elementwise gate chain runs on the ACT engine using the
  per-partition `scale` operand to fuse the multiplies:
      sig_io = Sigmoid([i|o])            (one op, two psum cols)
      t_g    = Tanh(g)
      t_c    = Tanh(t_g * sig_i)         (scale = sig_i)
      h      = Copy(t_c * sig_o)         (scale = sig_o)
- mean/log_std/v are one fused [65,3] matmul against a column holding
  [z(0:32) | u(32:48) | 0 | 1].
- Weights stream in three chunked DMAs so layer 0 starts as early as
  possible; the ACT table load (sigmoid set) is triggered right after the
  branch by a dependency-free warm-up op (scale=0.0 -> reads no real data).
- Head relus and the result copy run on DVE; everything else elementwise
  stays on ACT.
"""

import numpy as np

import concourse.bass as bass
from concourse import mybir
from concourse.bass_utils import run_bass_kernel_spmd

F32 = mybir.dt.float32
F32R = mybir.dt.float32r
AF = mybir.ActivationFunctionType

USE_F32R = True    # single-pass FP22-truncated PE matmuls (2x fewer PE ops)

H = 64          # hidden size
K = H + 1       # contraction dim: hidden + bias row
L = 500_000     # full input length
LIVE_CORE = 7   # shard owning x[L-1] under the natural row split

# column map inside the packed tensor wp [65, 1040]
_COL_X = 0                 # stage-0 rhs for the K=2 layer-0 matmuls: [x, 1]
_COL_L0 = 1                # layer 0 lhsT, 2 rows only: row0 = w, row1 = bias
_COL_H = 200               # h1..h5 rhs templates (cols 200..204, 1.0 in row 64)
_COL_V = 205               # z/u rhs template (cols 206,207 = pad)
_COL_L1 = 208              # layers 1..4 (4 x 192 cols)
_COL_FC = 208 + 4 * 192    # 976
_COL_C1 = _COL_FC + 32     # 1008
_COL_FH = _COL_C1 + 16     # 1024  fused head [mean, ls, v]; ends 1027
_WP_COLS = 1040

_CHUNK0 = 193              # rows 0:2, cols 0:193: x rhs + layer-0 lhsT (tiny)
_CHUNK1 = 208 + 2 * 192    # cols 200:592: rhs templates + L1, L2
# chunk2: cols 592:1027    L3, L4, heads

_CACHE = {}


def _pack_weights(inputs):
    """Pack all lhsT blocks: rows 0:64 = W.T, row 64 = bias."""
    wp = np.zeros((K, _WP_COLS), np.float32)

    def put(col, w_t, bias, row0=0):
        wp[row0 : row0 + w_t.shape[0], col : col + w_t.shape[1]] = w_t
        wp[H, col : col + w_t.shape[1]] = bias

    # LSTM layers, gate block order (i, o, g); f is dead.
    # Layer 0 has input_size=1, so its lhsT needs only 2 contraction rows
    # (row0 = w, row1 = bias) against the rhs [x, 1].
    w0 = np.asarray(inputs["Wih0"], np.float32)               # [256, 1]
    b0 = np.asarray(inputs["bih0"], np.float32) + np.asarray(
        inputs["bhh0"], np.float32
    )
    for gi, rows in enumerate((slice(0, 64), slice(192, 256), slice(128, 192))):
        wp[0, _COL_L0 + gi * 64 : _COL_L0 + gi * 64 + 64] = w0[rows, 0]
        wp[1, _COL_L0 + gi * 64 : _COL_L0 + gi * 64 + 64] = b0[rows]
    for l in range(1, 5):
        w = np.asarray(inputs["Wih"][l - 1], np.float32)      # [256, 64]
        b = np.asarray(inputs["bih"][l - 1], np.float32) + np.asarray(
            inputs["bhh"][l - 1], np.float32
        )
        base = _COL_L1 + (l - 1) * 192
        for gi, rows in enumerate((slice(0, 64), slice(192, 256), slice(128, 192))):
            put(base + gi * 64, w[rows].T, b[rows])

    put(_COL_FC, np.asarray(inputs["fc_w"], np.float32).T,
        np.asarray(inputs["fc_b"], np.float32))
    put(_COL_C1, np.asarray(inputs["c1_w"], np.float32).T,
        np.asarray(inputs["c1_b"], np.float32))
    # fused head: col0 mean (rows 0:32), col1 ls (rows 0:32), col2 v (rows 32:48)
    put(_COL_FH, np.asarray(inputs["mean_w"], np.float32).T,
        np.asarray(inputs["mean_b"], np.float32))
    put(_COL_FH + 1, np.asarray(inputs["ls_w"], np.float32).T,
        np.asarray(inputs["ls_b"], np.float32))
    put(_COL_FH + 2, np.asarray(inputs["c2_w"], np.float32).T,
        np.asarray(inputs["c2_b"], np.float32), row0=32)

    # rhs templates: zeros with the bias-partner 1.0 in row 64
    wp[1, _COL_X] = 1.0                # layer-0 rhs is [x, 1] (K=2)
    wp[H, _COL_H : _COL_V + 1] = 1.0   # cols 206,207 stay zero (pad)
    return wp


def _build_program():
    nc = bass.Bass()
    wp_d = nc.declare_dram_parameter("wp", [K, _WP_COLS], F32, isOutput=False)
    flag_d = nc.declare_dram_parameter("flag", [1, 1], mybir.dt.uint32,
                                       isOutput=False)
    out_d = nc.declare_dram_parameter("out", [3, 1], F32, isOutput=True)

    NW = _COL_FH + 3  # 1019 columns DMA'd

    with (
        nc.sbuf_tensor("WALL", [K, NW], F32) as WALL,
        nc.sbuf_tensor("warm", [1, 2], F32) as warm,
        nc.sbuf_tensor("res", [3, 1], F32) as res,
        nc.psum_tensor("PS", [H, 40], F32) as PS,  # 5x6 gate cols + fc, c1, head
        nc.sbuf_tensor("A", [H, 4], F32) as A,     # sig_i, sig_o, tanh_g, tanh_c
        nc.semaphore("dsem") as dsem,
        nc.semaphore("csem") as csem,
    ):
        # Per-core liveness branch: only the core whose flag == 1 (core 7,
        # the shard owning the last row) runs the compute body.  The flag
        # loads run in parallel on every engine; dead cores jump straight
        # to the program epilogue.
        regs = nc.alloc_registers("liveflag", engines=mybir.ALL_ENGINES)
        nc.regs_load(regs, flag_d[0:1, 0:1])

        with nc.If_cmp(regs, 1, "IS_EQ"):
            # ensure every engine (incl. GpSimd, which only appears in the
            # Block-exit barrier) has an instruction in the branch entry bb
            # so the If emits a branch for it
            nc.gpsimd.memset(warm[0:1, 0:1], 0.0)

            with nc.Block() as block:
                w = [WALL[0:2, _COL_L0 : _COL_L0 + 192]] + [
                    WALL[:, _COL_L1 + l * 192 : _COL_L1 + (l + 1) * 192]
                    for l in range(4)
                ]

                def rhs_col(c):
                    return WALL[:, c : c + 1]

                def mm(out, lhsT, rhs):
                    # fp32r (single-pass FP22) needs N even: rhs/out span 2
                    # columns, the second column is a discarded dummy
                    if USE_F32R:
                        lhsT = lhsT.bitcast(F32R)
                        rhs = rhs.bitcast(F32R)
                    return nc.tensor.matmul(out, lhsT, rhs, start=True, stop=True)

                def dma_cast(ap):
                    # fp32r matmuls require their producers (incl. DMA) to be
                    # marked as rounding to fp32r; bits are unchanged
                    return ap.bitcast(F32R) if USE_F32R else ap

                @block.sync
                def _(sync):
                    # chunk0: x rhs + layer-0 lhsT, rows 0:2 only (1.5 KB)
                    sync.dma_start(out=dma_cast(WALL[0:2, :_CHUNK0]),
                                   in_=dma_cast(wp_d[0:2, :_CHUNK0])).then_inc(dsem, 16)
                    # chunk1: rhs templates + L1, L2
                    sync.dma_start(
                        out=dma_cast(WALL[:, _COL_H:_CHUNK1]),
                        in_=dma_cast(wp_d[:, _COL_H:_CHUNK1]),
                    ).then_inc(dsem, 16)
                    sync.dma_start(
                        out=dma_cast(WALL[:, _CHUNK1:NW]),
                        in_=dma_cast(wp_d[:, _CHUNK1:NW]),
                    ).then_inc(dsem, 16)
                    sync.wait_ge(csem, 21)
                    sync.dma_start(out=out_d[:, :], in_=res[:, :]).then_inc(dsem, 16)

                @block.tensor
                def _(pe):
                    for l in range(5):
                        if l == 0:
                            pe.wait_ge(dsem, 16)              # chunk0
                        else:
                            if l == 1:
                                pe.wait_ge(dsem, 32)
                            elif l == 3:
                                pe.wait_ge(dsem, 48)
                            pe.wait_ge(csem, 3 * l)           # h_l ready
                        if l == 0:
                            rhs = WALL[0:2, _COL_X : _COL_X + 2]
                        else:
                            rhs = WALL[:, _COL_H + l - 1 : _COL_H + l + 1]
                        ps = PS[:, 6 * l : 6 * l + 6]
                        mm(ps[:, 0:2], w[l][:, 0:64], rhs)                       # i
                        mm(ps[:, 2:4], w[l][:, 64:128], rhs).then_inc(csem, 1)   # o -> 3l+1
                        mm(ps[:, 4:6], w[l][:, 128:192], rhs).then_inc(csem, 1)  # g -> 3l+2
                    pe.wait_ge(csem, 15)                      # h5 ready
                    mm(PS[0:32, 30:32], WALL[:, _COL_FC : _COL_FC + 32],
                       WALL[:, _COL_H + 4 : _COL_H + 6]).then_inc(csem, 1)       # 16 (fc)
                    pe.wait_ge(csem, 17)                      # z ready
                    # c1 writes partitions 32:48 -> fp32r needs start_partition 0, keep f32
                    nc.tensor.matmul(PS[32:48, 32:33],
                                     WALL[:, _COL_C1 : _COL_C1 + 16].bitcast(F32),
                                     rhs_col(_COL_V).bitcast(F32), start=True,
                                     stop=True).then_inc(csem, 1)                # 18 (c1)
                    pe.wait_ge(csem, 19)                      # u ready
                    mm(PS[0:3, 34:36], WALL[:, _COL_FH : _COL_FH + 3],
                       WALL[:, _COL_V : _COL_V + 2]).then_inc(csem, 1)           # 20 (head)

                @block.scalar
                def _(act):
                    # dependency-free warm-up: triggers the sigmoid/tanh table
                    # load right after the branch; scale=0.0 zeroes the
                    # (uninitialized) input
                    nc.scalar.activation(warm[0:1, 1:2], warm[0:1, 0:1],
                                         AF.Sigmoid, scale=0.0)
                    for l in range(5):
                        ps = PS[:, 6 * l : 6 * l + 6]
                        act.wait_ge(csem, 3 * l + 1)          # i, o landed; overlaps g matmul
                        nc.scalar.activation(A[:, 0:2], ps[:, 0:4:2], AF.Sigmoid)  # sig(i), sig(o)
                        act.wait_ge(csem, 3 * l + 2)          # g landed
                        nc.scalar.activation(A[:, 2:3], ps[:, 4:5], AF.Tanh)       # tanh(g)
                        nc.scalar.activation(A[:, 3:4], A[:, 2:3], AF.Tanh,
                                             scale=A[:, 0:1])                    # tanh(c)
                        if l == 0:
                            # chunk1's DMA writes the h-template cols; make sure
                            # it has landed before the first h write
                            act.wait_ge(dsem, 32)
                        h_out = WALL[0:64, _COL_H + l : _COL_H + l + 1]
                        if USE_F32R:
                            # consumed by an fp32r matmul: round-to-fp32r on write
                            h_out = h_out.bitcast(F32R)
                        nc.scalar.activation(h_out, A[:, 3:4], AF.Copy,
                                             scale=A[:, 1:2]).then_inc(csem, 1)  # 3l+3

                @block.vector
                def _(dve):
                    def vcol(p0, p1):
                        ap = WALL[p0:p1, _COL_V : _COL_V + 1]
                        # rows 0:48 of the V col feed the fp32r head matmul
                        return ap.bitcast(F32R) if USE_F32R else ap

                    dve.wait_ge(csem, 16)
                    nc.vector.tensor_relu(vcol(0, 32),
                                          PS[0:32, 30:31]).then_inc(csem, 1)     # 17 (z)
                    dve.wait_ge(csem, 18)
                    nc.vector.tensor_relu(vcol(32, 48),
                                          PS[32:48, 32:33]).then_inc(csem, 1)    # 19 (u)
                    dve.wait_ge(csem, 20)
                    nc.vector.tensor_copy(res[:, :], PS[0:3, 34:35]).then_inc(csem, 1)  # 21

        nc.end_ifs()

    return nc


def kernel(**inputs):
    if "nc" not in _CACHE:
        _CACHE["nc"] = _build_program()
    nc = _CACHE["nc"]

    wp = _pack_weights(inputs)
    wp[0, _COL_X] = np.float32(np.asarray(inputs["x"])[L - 1])

    in_maps = [
        {
            "wp": wp,
            "flag": np.array([[1 if c == LIVE_CORE else 0]], dtype=np.uint32),
        }
        for c in range(8)
    ]
    res = run_bass_kernel_spmd(nc, in_maps, list(range(8)))
    out = np.asarray(res.results[LIVE_CORE]["out"], np.float32)  # [3, 1]
    return (out[0:1, :], out[1:2, :], out[2:3, :])


# revision 28
# speedup vs baseline: 1.0142x; 1.0142x over previous
"""Trainium2 Bass kernel for nn_Net_60413009985719.

Reference semantics: x[L] -> 5 stacked single-step LSTM cells (seq_len=1,
zero initial (h, c)) applied independently to every "batch" row, then the
head reads ONLY h[-1:].  Because h_prev = c_prev = 0, rows never interact:
the output depends solely on the scalar x[L-1].

Sharding: data-parallel over L across the 8 cores, as per the spec hint --
"only the shard owning the last row" has live work.  Core 7 owns
x[437500:500000] under the natural row split, so core 7 runs the whole
5-cell + MLP-head chain; cores 0..6 hold dead rows (their h values are
never read by the head) and branch straight to the program end.  Each
core receives a per-core `flag` input (1 only on core 7) and the whole
compute body sits inside an `If(flag == 1)` branch.

Live-core implementation:
- Every matvec is a K=65 PE matmul with the bias folded in as an extra
  contraction row against a constant 1.0 in the rhs vector.  The f-gate is
  dead (f * c_prev == 0) and is never computed.
- The whole elementwise gate chain runs on the ACT engine using the
  per-partition `scale` operand to fuse the multiplies:
      sig_io = Sigmoid([i|o])            (one op, two psum cols)
      t_g    = Tanh(g)
      t_c    = Tanh(t_g * sig_i)         (scale = sig_i)
      h      = Copy(t_c * sig_o)         (scale = sig_o)
  No DVE at all in the layer chain -> no extra cross-engine hops.
- mean/log_std/v are one fused [65,3] matmul against a column holding
  [z(0:32) | u(32:48) | 0 | 1]; u lands at partition 32 via the matmul
  start-partition capability, so no cross-partition moves are needed.
- Weights stream in three chunked DMAs so layer 0 starts as early as
  possible; the ACT table load (sigmoid set) is triggered right after the
  branch by a dependency-free warm-up op (scale=0.0 -> reads no real data).
- Head relus and the result copy run on DVE (shorter op duration than
  ACT); everything else elementwise stays on ACT.
- Raw Bass with two semaphores (dma + one interleaved PE/ACT/DVE chain
  sem); the chain is serial, so standalone waits with transitivity
  suffice.
"""

import numpy as np

import concourse.bass as bass
from concourse import mybir
from concourse.bass_utils import run_bass_kernel_spmd

F32 = mybir.dt.float32
F32R = mybir.dt.float32r
AF = mybir.ActivationFunctionType

USE_F32R = False   # single-pass FP22-truncated PE matmuls (2x fewer PE ops)

H = 64          # hidden size
K = H + 1       # contraction dim: hidden + bias row
L = 500_000     # full input length
LIVE_CORE = 7   # shard owning x[L-1] under the natural row split

# column map inside the packed tensor wp [65, 1024]
_COL_X = 0                 # stage-0 rhs: [x, 0...0, 1]
_COL_L0 = 1                # layer 0 (192 cols: gate blocks i, o, g)
_COL_H = 193               # h1..h5 rhs templates (5 cols)
_COL_V = 198               # z/u rhs template (1 col; col 199 = pad)
_COL_L1 = 200              # layers 1..4 (4 x 192 cols)
_COL_FC = 200 + 4 * 192    # 968
_COL_C1 = _COL_FC + 32     # 1000
_COL_FH = _COL_C1 + 16     # 1016  fused head [mean, ls, v]; ends 1019
_WP_COLS = 1024

_CHUNK1 = 200              # cols 0:200   x, L0, rhs templates
_CHUNK2 = 200 + 2 * 192    # cols 200:584 L1, L2
# chunk3: cols 584:1019    L3, L4, heads

_CACHE = {}


def _pack_weights(inputs):
    """Pack all lhsT blocks: rows 0:64 = W.T, row 64 = bias."""
    wp = np.zeros((K, _WP_COLS), np.float32)

    def put(col, w_t, bias, row0=0):
        wp[row0 : row0 + w_t.shape[0], col : col + w_t.shape[1]] = w_t
        wp[H, col : col + w_t.shape[1]] = bias

    # LSTM layers, gate block order (i, o, g); f is dead.
    for l in range(5):
        if l == 0:
            w = np.asarray(inputs["Wih0"], np.float32)        # [256, 1]
            b = np.asarray(inputs["bih0"], np.float32) + np.asarray(
                inputs["bhh0"], np.float32
            )
        else:
            w = np.asarray(inputs["Wih"][l - 1], np.float32)  # [256, 64]
            b = np.asarray(inputs["bih"][l - 1], np.float32) + np.asarray(
                inputs["bhh"][l - 1], np.float32
            )
        base = _COL_L0 if l == 0 else _COL_L1 + (l - 1) * 192
        for gi, rows in enumerate((slice(0, 64), slice(192, 256), slice(128, 192))):
            put(base + gi * 64, w[rows].T, b[rows])

    put(_COL_FC, np.asarray(inputs["fc_w"], np.float32).T,
        np.asarray(inputs["fc_b"], np.float32))
    put(_COL_C1, np.asarray(inputs["c1_w"], np.float32).T,
        np.asarray(inputs["c1_b"], np.float32))
    # fused head: col0 mean (rows 0:32), col1 ls (rows 0:32), col2 v (rows 32:48)
    put(_COL_FH, np.asarray(inputs["mean_w"], np.float32).T,
        np.asarray(inputs["mean_b"], np.float32))
    put(_COL_FH + 1, np.asarray(inputs["ls_w"], np.float32).T,
        np.asarray(inputs["ls_b"], np.float32))
    put(_COL_FH + 2, np.asarray(inputs["c2_w"], np.float32).T,
        np.asarray(inputs["c2_b"], np.float32), row0=32)

    # rhs templates: zeros with the bias-partner 1.0 in row 64
    wp[H, _COL_X] = 1.0
    wp[H, _COL_H : _COL_V + 1] = 1.0   # col 199 stays zero (pad)
    return wp


def _build_program():
    nc = bass.Bass()
    wp_d = nc.declare_dram_parameter("wp", [K, _WP_COLS], F32, isOutput=False)
    flag_d = nc.declare_dram_parameter("flag", [1, 1], mybir.dt.uint32,
                                       isOutput=False)
    out_d = nc.declare_dram_parameter("out", [3, 1], F32, isOutput=True)

    NW = _COL_FH + 3  # 1019 columns DMA'd

    with (
        nc.sbuf_tensor("WALL", [K, NW], F32) as WALL,
        nc.sbuf_tensor("A", [H, 4], F32) as A,     # sig_i, sig_o, tanh_g, tanh_c
        nc.sbuf_tensor("warm", [1, 2], F32) as warm,
        nc.sbuf_tensor("res", [3, 1], F32) as res,
        nc.psum_tensor("PS", [H, 40], F32) as PS,  # 5x6 gate cols + fc, c1, head
        nc.semaphore("dsem") as dsem,
        nc.semaphore("csem") as csem,
    ):
        # Per-core liveness branch: only the core whose flag == 1 (core 7,
        # the shard owning the last row) runs the compute body.  The flag
        # loads run in parallel on every engine; dead cores jump straight
        # to the program epilogue.
        regs = nc.alloc_registers("liveflag", engines=mybir.ALL_ENGINES)
        nc.regs_load(regs, flag_d[0:1, 0:1])

        with nc.If_cmp(regs, 1, "IS_EQ"):
            # ensure every engine (incl. GpSimd, which only appears in the
            # Block-exit barrier) has an instruction in the branch entry bb
            # so the If emits a branch for it
            nc.gpsimd.memset(warm[0:1, 0:1], 0.0)

            with nc.Block() as block:
                w = [WALL[:, _COL_L0 : _COL_L0 + 192]] + [
                    WALL[:, _COL_L1 + l * 192 : _COL_L1 + (l + 1) * 192]
                    for l in range(4)
                ]

                def rhs_col(c):
                    return WALL[:, c : c + 1]

                def mm(out, lhsT, rhs):
                    # fp32r (single-pass FP22) needs N even: rhs/out span 2
                    # columns, the second column is a discarded dummy
                    if USE_F32R:
                        lhsT = lhsT.bitcast(F32R)
                        rhs = rhs.bitcast(F32R)
                    return nc.tensor.matmul(out, lhsT, rhs, start=True, stop=True)

                @block.sync
                def _(sync):
                    sync.dma_start(out=WALL[:, :_CHUNK1],
                                   in_=wp_d[:, :_CHUNK1]).then_inc(dsem, 16)
                    sync.dma_start(
                        out=WALL[:, _CHUNK1:_CHUNK2], in_=wp_d[:, _CHUNK1:_CHUNK2]
                    ).then_inc(dsem, 16)
                    sync.dma_start(
                        out=WALL[:, _CHUNK2:NW], in_=wp_d[:, _CHUNK2:NW]
                    ).then_inc(dsem, 16)
                    sync.wait_ge(csem, 21)
                    sync.dma_start(out=out_d[:, :], in_=res[:, :]).then_inc(dsem, 16)

                @block.tensor
                def _(pe):
                    for l in range(5):
                        if l == 0:
                            pe.wait_ge(dsem, 16)
                        else:
                            if l == 1:
                                pe.wait_ge(dsem, 32)
                            elif l == 3:
                                pe.wait_ge(dsem, 48)
                            pe.wait_ge(csem, 3 * l)           # h_l ready
                        c0 = _COL_X if l == 0 else _COL_H + l - 1
                        rhs = WALL[:, c0 : c0 + 2]
                        ps = PS[:, 6 * l : 6 * l + 6]
                        mm(ps[:, 0:2], w[l][:, 0:64], rhs)                       # i
                        mm(ps[:, 2:4], w[l][:, 64:128], rhs).then_inc(csem, 1)   # o -> 3l+1
                        mm(ps[:, 4:6], w[l][:, 128:192], rhs).then_inc(csem, 1)  # g -> 3l+2
                    pe.wait_ge(csem, 15)                      # h5 ready
                    mm(PS[0:32, 30:32], WALL[:, _COL_FC : _COL_FC + 32],
                       WALL[:, _COL_H + 4 : _COL_H + 6]).then_inc(csem, 1)       # 16 (fc)
                    pe.wait_ge(csem, 17)                      # z ready
                    # c1 writes partitions 32:48 -> fp32r needs start_partition 0, keep f32
                    nc.tensor.matmul(PS[32:48, 32:33],
                                     WALL[:, _COL_C1 : _COL_C1 + 16].bitcast(F32),
                                     rhs_col(_COL_V).bitcast(F32), start=True,
                                     stop=True).then_inc(csem, 1)                # 18 (c1)
                    pe.wait_ge(csem, 19)                      # u ready
                    mm(PS[0:3, 34:36], WALL[:, _COL_FH : _COL_FH + 3],
                       WALL[:, _COL_V : _COL_V + 2]).then_inc(csem, 1)           # 20 (head)

                @block.scalar
                def _(act):
                    # dependency-free warm-up: triggers the sigmoid/tanh table
                    # load right after the branch; scale=0.0 zeroes the
                    # (uninitialized) input
                    nc.scalar.activation(warm[0:1, 1:2], warm[0:1, 0:1],
                                         AF.Sigmoid, scale=0.0)
                    for l in range(5):
                        ps = PS[:, 6 * l : 6 * l + 6]
                        act.wait_ge(csem, 3 * l + 1)          # i, o landed; overlaps g matmul
                        nc.scalar.activation(A[:, 0:2], ps[:, 0:4:2], AF.Sigmoid)  # sig(i), sig(o)
                        act.wait_ge(csem, 3 * l + 2)          # g landed
                        nc.scalar.activation(A[:, 2:3], ps[:, 4:5], AF.Tanh)       # tanh(g)
                        nc.scalar.activation(A[:, 3:4], A[:, 2:3], AF.Tanh,
                                             scale=A[:, 0:1])                    # tanh(c)
                        nc.scalar.activation(WALL[0:64, _COL_H + l : _COL_H + l + 1],
                                             A[:, 3:4], AF.Copy,
                                             scale=A[:, 1:2]).then_inc(csem, 1)  # 3l+3

                @block.vector
                def _(dve):
                    dve.wait_ge(csem, 16)
                    nc.vector.tensor_relu(WALL[0:32, _COL_V : _COL_V + 1],
                                          PS[0:32, 30:31]).then_inc(csem, 1)     # 17 (z)
                    dve.wait_ge(csem, 18)
                    nc.vector.tensor_relu(WALL[32:48, _COL_V : _COL_V + 1],
                                          PS[32:48, 32:33]).then_inc(csem, 1)    # 19 (u)
                    dve.wait_ge(csem, 20)
                    nc.vector.tensor_copy(res[:, :], PS[0:3, 34:35]).then_inc(csem, 1)  # 21

        nc.end_ifs()

    return nc


def kernel(**inputs):
    if "nc" not in _CACHE:
        _CACHE["nc"] = _build_program()
    nc = _CACHE["nc"]

    wp = _pack_weights(inputs)
    wp[0, _COL_X] = np.float32(np.asarray(inputs["x"])[L - 1])

    in_maps = [
        {
            "wp": wp,
            "flag": np.array([[1 if c == LIVE_CORE else 0]], dtype=np.uint32),
        }
        for c in range(8)
    ]
    res = run_bass_kernel_spmd(nc, in_maps, list(range(8)))
    out = np.asarray(res.results[LIVE_CORE]["out"], np.float32)  # [3, 1]
    return (out[0:1, :], out[1:2, :], out[2:3, :])
